# revision 35
# baseline (speedup 1.0000x reference)
"""Trainium2 Bass kernel for nn_DGBasedVonMisesFisherKLD.

Computes okl = mean_j [ logsumexp_i (log_C_kappa + kappa * mu_n[i]@z2[j]) - log A ] - log_C_zero
where mu_n is row-normalized mu [2048, 32], z2 is z reshaped to [65536, 32].

Strategy (per spec sharding hint): shard the j axis (65536) across 8 cores,
mu replicated. Host pre-normalizes mu and folds kappa (muS = kappa*mu_n^T)
and does the final ln over the 65536 per-j sums; the device does the heavy
part only: for its 8192 j's,
    S_j = sum_i exp(kappa*m_ij - kappa)   (constant shift is safe: m <= 1)
Pipeline per 128-j tile:
  TensorE: 4x row-group-packed bf16 matmuls (K=32 quadrant packing)
           -> PSUM [128, 2048], double-buffered (2 tiles = all 8 banks)
  exp + sum over the 2048 i's, split 43/21 between ScalarE (native Exp with
           fused accumulate; ~2.2us/tile) and VectorE (custom DVE op pair:
           exp(y) ~ (1+t+t^2/2)^512, t=y/512, chained squarings with fused
           ADD accumulate; ~4.7us/tile) — the 21-tile spacing-3 pattern is
           the optimum of a schedule simulation under the 2-deep PSUM
           coupling
  acc[128, 64] -> HBM; host does ln + mean + constants.
First wave: one packed DMA per strip ([muS_g | zT cols 0:512], from pack0)
across the sync/gpsimd/scalar queues so the loop starts right after the
fixed ~6.5us NEFF preamble; the rest of zT streams behind the loop in
1024-column chunks.
No Ln and no normalization on device, so only one ACT table set is ever
loaded (warmed during the DMA ramp).
"""

import math
import os
import sys

import numpy as np

if "/opt/trn_rl_repo" not in sys.path:
    sys.path.insert(0, "/opt/trn_rl_repo")

BATCH = 2048
DIM = 32
N_SAMPLES = 32
N_CORES = 8
J_PER_CORE = BATCH * N_SAMPLES // N_CORES  # 8192
N_JT = J_PER_CORE // 128  # 64 j-tiles of 128
I_CHUNK = 512
N_IC = BATCH // I_CHUNK  # 4 i-chunks of 512
ZCH0 = 512  # zT columns in the packed first wave (tiles 0-3)
# streaming chunk boundaries for the rest of zT
Z_CHUNKS = [(ZCH0 + 1024 * k, ZCH0 + 1024 * (k + 1)) for k in range(7)] + [
    (ZCH0 + 7168, J_PER_CORE)
]

# which j-tiles are reduced on VectorE (custom exp) instead of ScalarE.
# 21 tiles at spacing 3 (plus 61) minimizes the simulated schedule under the
# 2-deep PSUM double-buffer coupling (local search over patterns).
DVE_MODE = int(os.environ.get("BASS_DVE_MODE", "1"))  # 0 = all-ScalarE
_env_tiles = os.environ.get("BASS_DVE_TILES", "")
if _env_tiles:
    DVE_TILES = tuple(int(x) for x in _env_tiles.split(",") if x)
else:
    DVE_TILES = tuple(list(range(0, 60, 3)) + [61])
# ACT tiles whose free-axis sum runs on the DMA engines (CCE accumulate into
# a stride-0 destination) instead of the ACT accumulator
_env_dma = os.environ.get("BASS_DMA_TILES", "")
if _env_dma:
    DMA_TILES = tuple(int(x) for x in _env_dma.split(",") if x) if _env_dma != "none" else ()
else:
    # default empty: the CCE stride-0 accumulate races across the parallel
    # DMA engines (lost updates), so DMA-side reduction is unusable
    DMA_TILES = ()
DMA_RED = 256  # reduce 2048 -> DMA_RED partials per j; host sums the rest

_CACHE = {}
_DVE_OPS = {}


# ---- fallback constants (normally passed in as inputs) ----
def _log_iv(v, x, n_terms=300):
    ks = np.arange(n_terms)
    lg = np.array([math.lgamma(k + 1.0) + math.lgamma(v + k + 1.0) for k in ks])
    logt = (v + 2 * ks) * np.log(x / 2.0) - lg
    m = logt.max()
    return float(m + np.log(np.exp(logt - m).sum()))


def _log_C_d(kappa, d):
    v = d / 2.0 - 1.0
    if kappa == 0.0:
        return float(math.lgamma(d / 2.0) - math.log(2.0) - (d / 2.0) * math.log(math.pi))
    return float(
        v * math.log(kappa) - (d / 2.0) * math.log(2.0 * math.pi) - _log_iv(v, kappa)
    )


def _register_dve_exp_ops():
    """Register two chained custom DVE ops computing exp(y + shift) for
    raw logits y = kappa*m in [-100, 100], shift = -kappa:
    op1: t = y*C0 + C2 (C0=1/512, C2=-kappa/512); u = 1 + t + t^2/2; u^4
    op2: (.)^128 (7 squarings) with fused ADD-reduction to accum_out.
    Result = (1 + t + t^2/2)^512 ~ exp(y-kappa), rel err ~ |y-k|^3/(6*512^2):
    ~1.4e-3 at the dominant logsumexp terms -> ~3e-5 relative on the final
    mean, fine for this loss."""
    if _DVE_OPS:
        return _DVE_OPS
    from concourse import dve_ops as DO
    from concourse.dve_spec import AluOp, C0, C1, C2, One, Spec, Src0, lower, sq
    from concourse.dve_uop import DveOpSpec

    t = Src0 * C0 + C2
    u = (One + t) + sq(t) * C1
    v = sq(sq(u))
    spec1 = Spec(
        body=v,
        reference=lambda in0, in1, c0, c1, c2: (
            1.0
            + (in0 * c0 + c2)
            + np.square(in0 * c0 + c2) * c1
        )
        ** 4,
    )

    w = Src0
    for _ in range(7):
        w = sq(w)
    spec2 = Spec(
        body=w,
        accum=AluOp.ADD,
        reference=lambda in0, in1, c0, c1, c2: (
            in0 ** 128,
            (in0 ** 128).sum(axis=-1, keepdims=True),
        ),
    )

    from concourse.dve_ops import has_src1

    ops = {}
    for name, spec in (("EXP_PT1_ANT", spec1), ("EXP_PT2_ANT", spec2)):
        if name in DO._SUB_OPCODE_FOR_NAME:
            ops[name] = next(o for o in DO.OPS if o.name == name)
            continue
        shas = {}
        for ver in ("v3", "v4"):
            try:
                s = DveOpSpec(
                    name=name,
                    opcode=DO._CUSTOM_DVE_ROW_BASE + len(DO.OPS),
                    uops=lower(spec, ver=ver),
                    rd1_en=has_src1(spec),
                )
                shas[ver] = s.sha(ver)
            except Exception:
                pass
        op = DO.DveOp(name, spec, subdim=False, uops_sha=shas)
        DO.OPS.append(op)
        DO._SUB_OPCODE_FOR_NAME[name] = (
            DO._CUSTOM_DVE_ROW_BASE + len(DO.OPS) - 1
        )
        DO.CUSTOM_DVE_SPECS[name] = spec
        ops[name] = op
    _DVE_OPS.update(ops)
    return _DVE_OPS


def _mm_dt(mybir, mm_dtype: str):
    return {
        "bf16": mybir.dt.bfloat16,
        "f32r": mybir.dt.float32r,
        "f32": mybir.dt.float32,
    }[mm_dtype]


def _build_nc(kappa: float, mm_dtype: str, dve_mode: int):
    """Build the single-core SPMD Bass program (same NEFF on all 8 cores)."""
    import concourse.tile as tile
    from concourse import bacc, mybir

    f32 = mybir.dt.float32
    mm_dt = _mm_dt(mybir, mm_dtype)
    AF = mybir.ActivationFunctionType

    if dve_mode:
        dve_ops = _register_dve_exp_ops()
        op1 = dve_ops["EXP_PT1_ANT"]
        op2 = dve_ops["EXP_PT2_ANT"]
    dve_tiles = set(DVE_TILES) if dve_mode else set()
    dma_tiles = [t for t in DMA_TILES if t not in dve_tiles]
    # exp output to SBUF scratch instead of in-place PSUM (probe: does the
    # in-place PSUM read+write slow the ACTIVATE?)
    act_out_sbuf = int(os.environ.get("BASS_ACT_OUT_SBUF", "0")) == 1

    nc = bacc.Bacc(
        "TRN2",
        target_bir_lowering=False,
        debug=False,
        num_devices=N_CORES,
        enable_partition_id=int(os.environ.get("BASS_PARTITION_ID", "0")) == 1,
        monotonic_sem_count=0,  # no remote_dma/collectives in this kernel
    )

    # pack0[:, 1536g:1536(g+1)] = [muS_g | zT chunk 0] for strip g, where
    # muS_g = (kappa * mu_n)^T[:, 512g:512(g+1)] (host-normalized) — one DMA
    # per strip covers everything tile 0..7 needs from that strip.
    pack_d = nc.dram_tensor(
        "pack0", [DIM, 4 * (I_CHUNK + ZCH0)], mm_dt, kind="ExternalInput"
    ).ap()
    # zT = z2^T [32, 8192]; chunks >= 1 stream from here into the 4 PE
    # row-group strip replicas for 4x-packed K=32 matmuls.
    zT_d = nc.dram_tensor("zT", [DIM, J_PER_CORE], mm_dt, kind="ExternalInput").ap()
    # acc[p, t] = sum_i expapprox(kappa*m - kappa) for j = t*128 + p
    out_d = nc.dram_tensor("out", [128, N_JT], f32, kind="ExternalOutput").ap()
    out2_d = None
    if dma_tiles:
        # DMA_RED partial sums per j for each DMA-reduced tile
        out2_d = nc.dram_tensor(
            "out2", [128, DMA_RED * len(dma_tiles)], f32, kind="ExternalOutput"
        ).ap()

    with tile.TileContext(nc) as tc:
        with (
            tc.tile_pool(name="big", bufs=1) as big,
            tc.tile_pool(name="small", bufs=1) as small,
            tc.tile_pool(name="scr", bufs=2) as scr,
        ):
            # ---- loads: zmu strip = [muS_g | zT strip g]; one packed DMA
            # per strip (from pack0) covers tiles 0..7, spread over the
            # sync/gpsimd/scalar queues; later zT chunks stream on
            # sync+gpsimd behind the loop.
            PW = I_CHUNK + ZCH0
            zmu = big.tile([128, I_CHUNK + J_PER_CORE], mm_dt)
            first_wave = (nc.sync, nc.gpsimd, nc.scalar, nc.sync)
            for g in range(4):
                first_wave[g].dma_start(
                    zmu[32 * g : 32 * (g + 1), 0:PW],
                    pack_d[:, g * PW : (g + 1) * PW],
                )
            qs = (nc.sync, nc.gpsimd)
            k = 0
            for lo, hi in Z_CHUNKS:
                for g in range(4):
                    qs[k % 2].dma_start(
                        zmu[32 * g : 32 * (g + 1), I_CHUNK + lo : I_CHUNK + hi],
                        zT_d[:, lo:hi],
                    )
                    k += 1

            bias_negk = small.tile([128, 1], f32)
            nc.vector.memset(bias_negk[:], -kappa)
            acc = small.tile([128, N_JT], f32)
            accd = None
            if dma_tiles:
                accd = small.tile([128, DMA_RED * len(dma_tiles)], f32)
                nc.vector.memset(accd[:], 0.0)

            # warm the exp ACT table set during the DMA ramp so the first
            # real ACTIVATE doesn't eat the ~1.3us PSEUDO_LOAD_ACT_FUNC_SET
            warm_act = small.tile([128, 1], f32)
            nc.scalar.activation(warm_act[:], bias_negk[:], AF.Exp)

            # ---- main loop ----
            idma = 0
            with tc.tile_pool(name="ps", bufs=2, space="PSUM") as ps:
                for t in range(N_JT):
                    P = ps.tile([128, BATCH], f32)
                    for g in range(4):
                        nc.tensor.matmul(
                            P[:, g * I_CHUNK : (g + 1) * I_CHUNK],
                            zmu[
                                32 * g : 32 * (g + 1),
                                I_CHUNK + t * 128 : I_CHUNK + (t + 1) * 128,
                            ],
                            zmu[32 * g : 32 * (g + 1), 0:I_CHUNK],
                            start=True,
                            stop=True,
                            tile_position=(32 * g, 0),
                        )
                    if t in dve_tiles:
                        # (tried: op1 split into [128,1024] halves to exploit
                        # region-level dep tracking — Vector gaps shrank 9us
                        # -> 5.4us but its own +157ns/tile overhead cancelled
                        # the gain; the consumer's LAST psum read still gates
                        # the next matmul, so the Scalar-side coupling gaps
                        # are unchanged. Kept unsplit for simplicity.)
                        s1 = scr.tile([128, BATCH], f32, tag="s1")
                        s2 = scr.tile([128, BATCH], f32, tag="s2")
                        nc.vector._custom_dve(
                            op1,
                            out=s1[:],
                            in0=P[:],
                            s0=1.0 / 512.0,
                            s1=0.5,
                            imm2=-float(kappa) / 512.0,
                        )
                        nc.vector._custom_dve(
                            op2,
                            out=s2[:],
                            in0=s1[:],
                            accum_out=acc[:, t : t + 1],
                        )
                    elif t in dma_tiles:
                        # exp to SBUF, then the DMA CCE reduces 2048 -> 16
                        # per partition (stride-0 dst accumulate); frees the
                        # ACT accumulator read and rides idle DMA engines
                        E = scr.tile([128, BATCH], f32, tag="e")
                        nc.scalar.activation(
                            E[:], P[:], AF.Exp, bias=bias_negk[:]
                        )
                        src = E[:].rearrange("p (s n) -> p s n", n=DMA_RED)
                        dst = (
                            accd[:, idma * DMA_RED : (idma + 1) * DMA_RED]
                            .unsqueeze(1)
                            .broadcast_to([128, BATCH // DMA_RED, DMA_RED])
                        )
                        nc.gpsimd.dma_start(
                            dst, src, accum_op=mybir.AluOpType.add
                        )
                        idma += 1
                    else:
                        if act_out_sbuf:
                            EO = scr.tile([128, BATCH], f32, tag="eo")
                            nc.scalar.activation(
                                EO[:],
                                P[:],
                                AF.Exp,
                                bias=bias_negk[:],
                                accum_out=acc[:, t : t + 1],
                            )
                        else:
                            nc.scalar.activation(
                                P[:],
                                P[:],
                                AF.Exp,
                                bias=bias_negk[:],
                                accum_out=acc[:, t : t + 1],
                            )

            # scalar issues the out DMA: it owns the last accumulator read,
            # so no cross-engine sem hop before the store
            nc.scalar.dma_start(out_d[:], acc[:])
            if dma_tiles:
                nc.gpsimd.dma_start(out2_d[:], accd[:])

    nc.finalize()  # Bacc passes: wait-splitting, nop-fusion, act table loads
    return nc


def _get_nc(kappa: float, mm_dtype: str, dve_mode: int = DVE_MODE):
    key = (kappa, mm_dtype, dve_mode)
    if key not in _CACHE:
        _CACHE[key] = _build_nc(kappa, mm_dtype, dve_mode)
    return _CACHE[key]


def _install_trace_hook():
    """The image's antenv lacks axon_hooks; shim it so trace=True can ship
    NTFFs back through libaxon_pjrt.so. Safe no-op on failure."""
    try:
        import types

        import antenv

        if "antenv.axon_hooks" not in sys.modules:
            mod = types.ModuleType("antenv.axon_hooks")
            mod._hook = None
            mod.set_axon_ntff_profile_hook = lambda h: setattr(mod, "_hook", h)
            mod.get_axon_ntff_profile_hook = lambda: mod._hook
            sys.modules["antenv.axon_hooks"] = mod
            antenv.axon_hooks = mod
        hooks = sys.modules["antenv.axon_hooks"]
        if hooks.get_axon_ntff_profile_hook() is None:
            from trn_agent_boot.trn_boot import _ntff_profile_via_ctypes

            hooks.set_axon_ntff_profile_hook(
                _ntff_profile_via_ctypes("/opt/axon/libaxon_pjrt.so")
            )
        return True
    except Exception as e:  # pragma: no cover
        print(f"trace hook install failed: {e}")
        return False


def _np_dt(mm_dtype: str):
    if mm_dtype == "bf16":
        import ml_dtypes

        return np.dtype(ml_dtypes.bfloat16)
    return np.dtype(np.float32)


def _run(mu, z, kappa, log_C_kappa, log_C_zero, n_samples, trace=False):
    from concourse.bass_utils import run_bass_kernel_spmd

    if trace:
        trace = _install_trace_hook()

    mu = np.asarray(mu, dtype=np.float32)
    z = np.ascontiguousarray(np.asarray(z, dtype=np.float32))
    B, d = mu.shape
    n = int(n_samples)
    assert (B, d, n) == (BATCH, DIM, N_SAMPLES), (B, d, n)

    mm_dtype = os.environ.get("BASS_MM_DTYPE", "bf16")
    nc = _get_nc(float(kappa), mm_dtype)
    np_dt = _np_dt(mm_dtype)

    # host-side input marshalling: normalize mu, fold kappa, transpose,
    # and pack [muS_g | zT chunk 0] per strip g for the one-DMA first wave
    mu_n = mu / np.linalg.norm(mu, axis=1, keepdims=True)
    muS = (float(kappa) * mu_n).T.astype(np_dt)  # [32, 2048]
    rows = B // N_CORES
    in_maps = []
    for c in range(N_CORES):
        zc = z[c * rows : (c + 1) * rows].reshape(-1, d)
        zT = np.ascontiguousarray(zc.T.astype(np_dt))  # [32, 8192]
        pack = np.empty((d, 4 * (I_CHUNK + ZCH0)), dtype=np_dt)
        PW = I_CHUNK + ZCH0
        for g in range(4):
            pack[:, g * PW : g * PW + I_CHUNK] = muS[
                :, g * I_CHUNK : (g + 1) * I_CHUNK
            ]
            pack[:, g * PW + I_CHUNK : (g + 1) * PW] = zT[:, 0:ZCH0]
        in_maps.append({"zT": zT, "pack0": pack})

    res = run_bass_kernel_spmd(
        nc, in_maps, core_ids=list(range(N_CORES)), trace=trace
    )
    # r["out"][p, t] = S_{j=t*128+p} with the exp(-kappa) shift folded in;
    # lse_j = ln(S_j) + kappa. okl = log_C_kappa + mean_j lse_j - ln(B)
    #                              - log_C_zero
    dve_set = set(DVE_TILES) if DVE_MODE else set()
    dma_tiles = [t for t in DMA_TILES if t not in dve_set]
    total = 0.0
    for r in res.results:
        s = r["out"].astype(np.float64)
        if dma_tiles:
            s2 = r["out2"].astype(np.float64)
            for k, t in enumerate(dma_tiles):
                s[:, t] = s2[:, k * DMA_RED : (k + 1) * DMA_RED].sum(axis=1)
        total += float(np.log(s).sum())
    okl = (
        float(log_C_kappa)
        + float(kappa)
        - math.log(B)
        - float(log_C_zero)
        + total / (B * n)
    )
    return np.float32(okl), res


def kernel(
    mu,
    z,
    kappa=100.0,
    log_C_kappa=None,
    log_C_zero=None,
    n_samples=N_SAMPLES,
    **_ignored,
):
    mu = np.asarray(mu)
    if log_C_kappa is None:
        log_C_kappa = _log_C_d(float(kappa), mu.shape[1])
    if log_C_zero is None:
        log_C_zero = _log_C_d(0.0, mu.shape[1])
    okl, _ = _run(mu, z, kappa, log_C_kappa, log_C_zero, n_samples, trace=False)
    return okl
